# revision 4
# baseline (speedup 1.0000x reference)
"""Trainium2 Bass kernel for nn_ActorCritic (GIN actor-critic, 8 disjoint graphs).

Sharding: graph b -> NeuronCore b (data parallel over the batch of disjoint
graphs). Each core holds its diagonal adjacency block (transposed, bf16),
its node features, and replicated MLP weights. The only cross-core traffic
is the 4 BatchNorm statistics exchanges, done as tiny AllGathers.

Self-contained: hardcodes all shapes. The host side only reshapes /
transposes / one-hot-encodes inputs (no model math on CPU).
"""

import numpy as np
import ml_dtypes

import concourse.bass as bass
import concourse.bacc as bacc
import concourse.mybir as mybir
import concourse.tile as tile
from concourse.bass_utils import run_bass_kernel_spmd

# ---- problem constants ----
B = 8            # graphs == cores
N = 2000         # nodes per graph
NJ = 100         # candidates per graph
HID = 64
HACT = 32
BN_EPS = 1e-5
CNT = float(B * N)          # batchnorm count (16000)
NEG_BIG = -1.0e30

NT = 16                      # node k-tiles of 128 (last has 80 rows)
K_LIST = [128] * 15 + [80]
CHUNKS = [(0, 512), (512, 512), (1024, 512), (1536, 464)]  # node columns

f32 = mybir.dt.float32
bf16 = mybir.dt.bfloat16

AX = mybir.AxisListType.X
ALU = mybir.AluOpType
ACT = mybir.ActivationFunctionType


def build_graph(bc2: float, reps: int = 1):
    """Build the SPMD graph (same program on all 8 cores)."""
    nc = bacc.Bacc("TRN2", target_bir_lowering=False, debug=False,
                   num_devices=B)

    adjT_e = nc.dram_tensor("adjT", [N, N], bf16, kind="ExternalInput")
    xT_e = nc.dram_tensor("xT", [2, N], f32, kind="ExternalInput")
    w1l1_e = nc.dram_tensor("w1l1", [2, HID], f32, kind="ExternalInput")
    w2l1_e = nc.dram_tensor("w2l1", [HID, HID], f32, kind="ExternalInput")
    w1l2_e = nc.dram_tensor("w1l2", [HID, HID], f32, kind="ExternalInput")
    w2l2_e = nc.dram_tensor("w2l2", [HID, HID], f32, kind="ExternalInput")
    gbe_e = nc.dram_tensor("gbe", [HID, 8], f32, kind="ExternalInput")
    paug_e = nc.dram_tensor("paug", [2048, NJ + 1], f32, kind="ExternalInput")
    actw_e = nc.dram_tensor("actw", [HID, 3 * HACT], f32, kind="ExternalInput")
    headw_e = nc.dram_tensor("headw", [HACT, 4], f32, kind="ExternalInput")
    maskval_e = nc.dram_tensor("maskval", [1, NJ], f32, kind="ExternalInput")
    ident_e = nc.dram_tensor("ident", [128, 128], f32, kind="ExternalInput")
    out_e = nc.dram_tensor("out", [1, NJ + 1], f32, kind="ExternalOutput")

    with tile.TileContext(nc) as tc:
        with (
            tc.tile_pool(name="sb", bufs=1) as sb,
            tc.tile_pool(name="ps", bufs=1, space="PSUM") as ps,
            tc.tile_pool(name="dr", bufs=1, space="DRAM") as dr,
        ):
            for rep in range(reps):
                _emit_one(nc, sb, ps, dr, rep,
                          adjT_e, xT_e, w1l1_e, w2l1_e, w1l2_e, w2l2_e,
                          gbe_e, paug_e, actw_e, headw_e, maskval_e, ident_e,
                          out_e, bc2)
    nc.compile()
    return nc


def _emit_one(nc, sb, ps, dr, rep,
              adjT_e, xT_e, w1l1_e, w2l1_e, w1l2_e, w2l2_e,
              gbe_e, paug_e, actw_e, headw_e, maskval_e, ident_e,
              out_e, bc2):
    # ---------------- SBUF residency + input DMAs ----------------
    adjT = sb.tile([128, NT * N], bf16, tag="adjT")
    # 7 groups of 2 row-tiles (1MB each) + tile14 + tile15 (ragged 80 rows)
    for g in range(7):
        nc.sync.dma_start(
            adjT[0:128, 4000 * g:4000 * (g + 1)].rearrange(
                "p (t n) -> p t n", n=N),
            adjT_e[256 * g:256 * (g + 1), :].rearrange(
                "(t p) n -> p t n", p=128))
    nc.sync.dma_start(adjT[0:128, 14 * N:15 * N], adjT_e[1792:1920, :])
    nc.sync.dma_start(adjT[0:80, 15 * N:16 * N], adjT_e[1920:2000, :])

    xT = sb.tile([2, N], f32, tag="xT")
    nc.sync.dma_start(xT[:, :], xT_e[:, :])
    w1l1 = sb.tile([2, HID], f32, tag="w1l1")
    nc.sync.dma_start(w1l1[:, :], w1l1_e[:, :])
    w2l1 = sb.tile([HID, HID], f32, tag="w2l1")
    nc.sync.dma_start(w2l1[:, :], w2l1_e[:, :])
    w1l2 = sb.tile([HID, HID], f32, tag="w1l2")
    nc.sync.dma_start(w1l2[:, :], w1l2_e[:, :])
    w2l2 = sb.tile([HID, HID], f32, tag="w2l2")
    nc.sync.dma_start(w2l2[:, :], w2l2_e[:, :])
    gbe = sb.tile([HID, 8], f32, tag="gbe")
    nc.sync.dma_start(gbe[:, :], gbe_e[:, :])
    paug = sb.tile([128, NT * (NJ + 1)], f32, tag="paug")
    nc.sync.dma_start(
        paug[:, :].rearrange("p (t j) -> p t j", j=NJ + 1),
        paug_e[:, :].rearrange("(t p) j -> p t j", p=128))
    actw = sb.tile([HID, 3 * HACT], f32, tag="actw")
    nc.sync.dma_start(actw[:, :], actw_e[:, :])
    headw = sb.tile([HACT, 4], f32, tag="headw")
    nc.sync.dma_start(headw[:, :], headw_e[:, :])
    maskval = sb.tile([1, NJ], f32, tag="maskval")
    nc.sync.dma_start(maskval[:, :], maskval_e[:, :])
    ident = sb.tile([128, 128], f32, tag="ident")
    nc.sync.dma_start(ident[:, :], ident_e[:, :])

    # shared scratch
    sq = sb.tile([HID, 2048], f32, tag="sq")          # squares scratch
    u1 = sb.tile([128, NT * HID], bf16, tag="u1")
    u2 = sb.tile([128, NT * HID], bf16, tag="u2")
    h1m = sb.tile([HID, N], f32, tag="h1m")
    h1 = sb.tile([HID, N], f32, tag="h1")
    h2m = sb.tile([HID, N], f32, tag="h2m")
    h2 = sb.tile([HID, N], f32, tag="h2")
    h2n = sb.tile([128, NT * HID], f32, tag="h2n")

    # ---------------- batchnorm block ----------------
    def bn_block(i, zp, out_sb):
        """z (psum [64, 0:N]) -> out_sb = relu((z - mean)*g*rsqrt(var+eps) + be)
        with mean/var over all 16000 nodes (cross-core AllGather)."""
        stats8 = sb.tile([HID, 8], f32, tag=f"st8_{i}")
        for j, (c0, ln) in enumerate(CHUNKS):
            nc.vector.reduce_sum(stats8[:, j:j + 1], zp[0:HID, c0:c0 + ln],
                                 axis=AX)
            nc.scalar.activation(sq[:, c0:c0 + ln], zp[0:HID, c0:c0 + ln],
                                 ACT.Square,
                                 accum_out=stats8[:, 4 + j:5 + j])
        stats2 = sb.tile([HID, 2], f32, tag=f"st2_{i}")
        nc.vector.reduce_sum(stats2[:, :],
                             stats8[:, :].rearrange("p (s f) -> p s f", s=2),
                             axis=AX)
        cc_in = dr.tile([HID, 2], f32, tag=f"ccin_{i}")
        cc_out = dr.tile([B, HID, 2], f32, tag=f"ccout_{i}")
        nc.sync.dma_start(cc_in[:, :], stats2[:, :])
        nc.gpsimd.collective_compute(
            "AllGather", ALU.bypass,
            replica_groups=[list(range(B))],
            ins=[cc_in.opt()], outs=[cc_out.opt()])
        ag = sb.tile([HID, B, 2], f32, tag=f"ag_{i}")
        nc.sync.dma_start(ag[:, :, :],
                          cc_out[:, :, :].rearrange("r p f -> p r f"))
        bnw = sb.tile([HID, 16], f32, tag=f"bnw_{i}")
        agf = ag[:, :, :].rearrange("p r f -> p (r f)")
        # tree-reduce 8 ranks -> (S, Q) at bnw[:, 12:14]
        nc.vector.tensor_add(bnw[:, 0:8], agf[:, 0:8], agf[:, 8:16])
        nc.vector.tensor_add(bnw[:, 8:12], bnw[:, 0:4], bnw[:, 4:8])
        nc.vector.tensor_add(bnw[:, 12:14], bnw[:, 8:10], bnw[:, 10:12])
        bnv = sb.tile([HID, 8], f32, tag=f"bnv_{i}")
        # (mean, meansq)
        nc.vector.tensor_scalar_mul(bnv[:, 0:2], bnw[:, 12:14], 1.0 / CNT)
        nc.scalar.square(bnv[:, 2:3], bnv[:, 0:1])                 # mean^2
        # var + eps = (meansq - mean^2) + eps
        nc.vector.tensor_scalar(bnv[:, 3:4], bnv[:, 1:2],
                                scalar1=bnv[:, 2:3], scalar2=BN_EPS,
                                op0=ALU.subtract, op1=ALU.add)
        nc.vector.reciprocal(bnv[:, 4:5], bnv[:, 3:4])
        nc.scalar.sqrt(bnv[:, 5:6], bnv[:, 4:5])                   # rsqrt
        g_col = gbe[:, 2 * i:2 * i + 1]
        be_col = gbe[:, 2 * i + 1:2 * i + 2]
        nc.vector.tensor_mul(bnv[:, 6:7], bnv[:, 5:6], g_col)      # scale
        nc.vector.tensor_mul(bnv[:, 7:8], bnv[:, 0:1], bnv[:, 6:7])  # m*scale
        nc.vector.tensor_sub(bnv[:, 7:8], be_col, bnv[:, 7:8])     # bias
        for c0, ln in CHUNKS:
            nc.scalar.activation(out_sb[:, c0:c0 + ln], zp[0:HID, c0:c0 + ln],
                                 ACT.Relu,
                                 bias=bnv[:, 7:8], scale=bnv[:, 6:7])

    # ---------------- u1 = x @ W1l1 (node-major, bf16) ----------------
    pa = ps.tile([128, 2048], f32, tag="pa")
    for k in range(NT):
        kk = K_LIST[k]
        nc.tensor.matmul(pa[0:kk, HID * k:HID * (k + 1)],
                         lhsT=xT[0:2, 128 * k:128 * k + kk],
                         rhs=w1l1[0:2, :], start=True, stop=True)
    nc.vector.tensor_copy(u1[:, 0:512], pa[0:128, 0:512])
    nc.vector.tensor_copy(u1[:, 512:1024], pa[0:128, 512:1024])

    # ---------------- layer 1: z1 = adj @ u1 ----------------
    pb = ps.tile([128, 2048], f32, tag="pb")
    for k in range(NT):
        kk = K_LIST[k]
        for (c0, ln) in CHUNKS:
            nc.tensor.matmul(pb[0:HID, c0:c0 + ln],
                             lhsT=u1[0:kk, HID * k:HID * (k + 1)],
                             rhs=adjT[0:kk, N * k + c0:N * k + c0 + ln],
                             start=(k == 0), stop=(k == NT - 1),
                             skip_group_check=True)
    bn_block(0, pb, h1m)

    # ---------------- rep2 = h1m @ W2l1 (channel-major) ----------------
    pa = ps.tile([128, 2048], f32, tag="pa")
    for (c0, ln) in CHUNKS:
        nc.tensor.matmul(pa[0:HID, c0:c0 + ln], lhsT=w2l1[:, :],
                         rhs=h1m[:, c0:c0 + ln], start=True, stop=True)
    bn_block(1, pa, h1)

    # ---------------- u2 = h1 @ W1l2 (node-major, bf16) ----------------
    pb = ps.tile([128, 2048], f32, tag="pb")
    for k in range(NT):
        kk = K_LIST[k]
        nc.tensor.matmul(pb[0:kk, HID * k:HID * (k + 1)],
                         lhsT=h1[0:HID, 128 * k:128 * k + kk],
                         rhs=w1l2[:, :], start=True, stop=True)
    nc.vector.tensor_copy(u2[:, 0:512], pb[0:128, 0:512])
    nc.vector.tensor_copy(u2[:, 512:1024], pb[0:128, 512:1024])

    # ---------------- layer 2: z2 = adj @ u2 ----------------
    pa = ps.tile([128, 2048], f32, tag="pa")
    for k in range(NT):
        kk = K_LIST[k]
        for (c0, ln) in CHUNKS:
            nc.tensor.matmul(pa[0:HID, c0:c0 + ln],
                             lhsT=u2[0:kk, HID * k:HID * (k + 1)],
                             rhs=adjT[0:kk, N * k + c0:N * k + c0 + ln],
                             start=(k == 0), stop=(k == NT - 1),
                             skip_group_check=True)
    bn_block(2, pa, h2m)

    # ---------------- rep2l2 = h2m @ W2l2 ----------------
    pb = ps.tile([128, 2048], f32, tag="pb")
    for (c0, ln) in CHUNKS:
        nc.tensor.matmul(pb[0:HID, c0:c0 + ln], lhsT=w2l2[:, :],
                         rhs=h2m[:, c0:c0 + ln], start=True, stop=True)
    bn_block(3, pb, h2)

    # ---------------- transpose h2 -> node-major ----------------
    pa = ps.tile([128, 2048], f32, tag="pa")
    for k in range(NT):
        kk = K_LIST[k]
        nc.tensor.transpose(pa[0:kk, HID * k:HID * (k + 1)],
                            h2[0:HID, 128 * k:128 * k + kk],
                            ident[0:HID, 0:HID])
    nc.vector.tensor_copy(h2n[:, 0:512], pa[0:128, 0:512])
    nc.vector.tensor_copy(h2n[:, 512:1024], pa[0:128, 512:1024])

    # ---------------- candidates + pooling: C = h2n^T @ paug ----------------
    pb = ps.tile([128, 2048], f32, tag="pb")
    for k in range(NT):
        kk = K_LIST[k]
        nc.tensor.matmul(pb[0:HID, 0:NJ + 1],
                         lhsT=h2n[0:kk, HID * k:HID * (k + 1)],
                         rhs=paug[0:kk, (NJ + 1) * k:(NJ + 1) * (k + 1)],
                         start=(k == 0), stop=(k == NT - 1))
    C = sb.tile([HID, NJ + 1], f32, tag="C")
    nc.vector.tensor_copy(C[:, :], pb[0:HID, 0:NJ + 1])

    # ---------------- actor / critic heads ----------------
    pb2 = ps.tile([128, 2048], f32, tag="pb")
    # t1 = Wa1top^T @ C  [32, 101] ; w = Wa1bot^T @ hp ; tcv = Wc1^T @ hp
    nc.tensor.matmul(pb2[0:HACT, 0:NJ + 1], lhsT=actw[:, 0:HACT],
                     rhs=C[:, :], start=True, stop=True)
    nc.tensor.matmul(pb2[0:HACT, 512:513], lhsT=actw[:, HACT:2 * HACT],
                     rhs=C[:, NJ:NJ + 1], start=True, stop=True)
    nc.tensor.matmul(pb2[0:HACT, 513:514], lhsT=actw[:, 2 * HACT:3 * HACT],
                     rhs=C[:, NJ:NJ + 1], start=True, stop=True)
    hw = sb.tile([HACT, 2], f32, tag="hw")
    # bias_a = w + ba1 ; (bc1 is headw[:, 3])
    nc.vector.tensor_add(hw[:, 0:1], pb2[0:HACT, 512:513], headw[:, 2:3])
    T = sb.tile([HACT, NJ + 1], f32, tag="T")
    nc.scalar.activation(T[:, 0:NJ], pb2[0:HACT, 0:NJ], ACT.Tanh,
                         bias=hw[:, 0:1])
    nc.scalar.activation(T[:, NJ:NJ + 1], pb2[0:HACT, 513:514], ACT.Tanh,
                         bias=headw[:, 3:4])
    # scores = Wa2^T @ T[:, :NJ] ; value = Wc2^T @ T[:, NJ]
    nc.tensor.matmul(pb2[0:1, 1024:1024 + NJ], lhsT=headw[:, 0:1],
                     rhs=T[:, 0:NJ], start=True, stop=True)
    nc.tensor.matmul(pb2[0:1, 1536:1537], lhsT=headw[:, 1:2],
                     rhs=T[:, NJ:NJ + 1], start=True, stop=True)
    sm = sb.tile([1, NJ], f32, tag="sm")
    nc.vector.tensor_add(sm[:, :], pb2[0:1, 1024:1024 + NJ], maskval[:, :])
    red = sb.tile([1, 4], f32, tag="red")
    nc.vector.reduce_max(red[:, 0:1], sm[:, :], axis=AX)
    nc.vector.tensor_scalar_mul(red[:, 1:2], red[:, 0:1], -1.0)
    e = sb.tile([1, NJ], f32, tag="e")
    nc.scalar.activation(e[:, :], sm[:, :], ACT.Exp, bias=red[:, 1:2],
                         accum_out=red[:, 2:3])
    nc.vector.reciprocal(red[:, 3:4], red[:, 2:3])
    pi = sb.tile([1, NJ], f32, tag="pi")
    nc.scalar.activation(pi[:, :], e[:, :], ACT.Copy, scale=red[:, 3:4])
    v = sb.tile([1, 1], f32, tag="v")
    nc.scalar.activation(v[:, :], pb2[0:1, 1536:1537],
                         ACT.Copy, bias=float(bc2))
    nc.sync.dma_start(out_e[0:1, 0:NJ], pi[:, :])
    nc.sync.dma_start(out_e[0:1, NJ:NJ + 1], v[:, :])


# ---------------- host side ----------------

def _prep_in_maps(x, graph_pool, adj, candidate, mask, params):
    x = np.asarray(x, dtype=np.float32)
    graph_pool = np.asarray(graph_pool, dtype=np.float32)
    adj = np.asarray(adj, dtype=np.float32)
    candidate = np.asarray(candidate).astype(np.int64)
    mask = np.asarray(mask)
    g0, g1 = params['gin'][0], params['gin'][1]
    a, c = params['actor'], params['critic']
    f = lambda t: np.ascontiguousarray(np.asarray(t, dtype=np.float32))

    w1l1 = f(g0['W1'])
    w2l1 = f(g0['W2'])
    w1l2 = f(g1['W1'])
    w2l2 = f(g1['W2'])
    gbe = np.stack([f(g0['g1']), f(g0['be1']), f(g0['g']), f(g0['be']),
                    f(g1['g1']), f(g1['be1']), f(g1['g']), f(g1['be'])],
                   axis=1)  # [64, 8]
    wa1 = f(a['W1'])                       # [128, 32]
    actw = np.concatenate([wa1[:HID], wa1[HID:], f(c['W1'])], axis=1)
    headw = np.stack([f(a['W2'])[:, 0], f(c['W2'])[:, 0],
                      np.broadcast_to(f(a['b1']), (HACT,)),
                      np.broadcast_to(f(c['b1']), (HACT,))], axis=1)
    ba2 = float(np.asarray(a['b2']).reshape(-1)[0])
    bc2 = float(np.asarray(c['b2']).reshape(-1)[0])
    ident = np.eye(128, dtype=np.float32)

    in_maps = []
    for b in range(B):
        sl = slice(b * N, (b + 1) * N)
        adjT = np.ascontiguousarray(adj[sl, sl].T).astype(ml_dtypes.bfloat16)
        xT = np.ascontiguousarray(x[sl].T)
        paug = np.zeros((2048, NJ + 1), dtype=np.float32)
        paug[candidate[b], np.arange(NJ)] = 1.0
        paug[0:N, NJ] = graph_pool[b, sl]
        maskval = (np.where(mask[b], NEG_BIG, 0.0) + ba2) \
            .astype(np.float32).reshape(1, NJ)
        in_maps.append({
            "adjT": adjT, "xT": xT, "w1l1": w1l1, "w2l1": w2l1,
            "w1l2": w1l2, "w2l2": w2l2, "gbe": gbe, "paug": paug,
            "actw": actw, "headw": headw, "maskval": maskval, "ident": ident,
        })
    return in_maps, bc2


def run(inputs: dict, reps: int = 1, nc=None):
    """Run on hardware; returns ((pi, v), nc) so callers can reuse the graph."""
    in_maps, bc2 = _prep_in_maps(
        inputs['x'], inputs['graph_pool'], inputs['adj'],
        inputs['candidate'], inputs['mask'], inputs['params'])
    if nc is None:
        nc = build_graph(bc2, reps=reps)
    res = run_bass_kernel_spmd(nc, in_maps, core_ids=list(range(B)))
    outs = [res.results[i]["out"] for i in range(B)]
    pi = np.stack([o[0, :NJ] for o in outs]).astype(np.float32)[:, :, None]
    v = np.stack([o[0, NJ:NJ + 1] for o in outs]).astype(np.float32)
    return (pi, v), nc


def kernel(x, graph_pool, padded_nei, adj, candidate, mask, params):
    (pi, v), _ = run({'x': x, 'graph_pool': graph_pool, 'adj': adj,
                      'candidate': candidate, 'mask': mask, 'params': params})
    return pi, v


# revision 5
# speedup vs baseline: 1.4583x; 1.4583x over previous
"""Trainium2 Bass kernel for nn_ActorCritic (GIN actor-critic, 8 disjoint graphs).

Sharding: graph b -> NeuronCore b (data parallel over the batch of disjoint
graphs). Each core holds its diagonal adjacency block (transposed, bf16),
its node features, and replicated MLP weights. The only cross-core traffic
is the 4 BatchNorm statistics exchanges, done as tiny AllGathers.

Self-contained: hardcodes all shapes. The host side only reshapes /
transposes / one-hot-encodes inputs (no model math on CPU).
"""

import numpy as np
import ml_dtypes

import concourse.bass as bass
import concourse.bacc as bacc
import concourse.mybir as mybir
import concourse.tile as tile
from concourse.bass_utils import run_bass_kernel_spmd

# ---- problem constants ----
B = 8            # graphs == cores
N = 2000         # nodes per graph
NJ = 100         # candidates per graph
HID = 64
HACT = 32
BN_EPS = 1e-5
CNT = float(B * N)          # batchnorm count (16000)
NEG_BIG = -1.0e30

NT = 16                      # node k-tiles of 128 (last has 80 rows)
K_LIST = [128] * 15 + [80]
CHUNKS = [(0, 512), (512, 512), (1024, 512), (1536, 464)]  # node columns

f32 = mybir.dt.float32
bf16 = mybir.dt.bfloat16

AX = mybir.AxisListType.X
ALU = mybir.AluOpType
ACT = mybir.ActivationFunctionType

# wpack column layout: [w2l1 64][w1l2 64][w2l2 64][gbe 8][eps 1][actw 96]
WP_W2L1 = 0
WP_W1L2 = 64
WP_W2L2 = 128
WP_GBE = 192
WP_EPS = 200
WP_ACT = 201
WP_COLS = 297


def build_graph(bc2: float, reps: int = 1):
    nc = bacc.Bacc("TRN2", target_bir_lowering=False, debug=False,
                   num_devices=B)

    adjT_e = nc.dram_tensor("adjT", [N, N], bf16, kind="ExternalInput")
    xn_e = nc.dram_tensor("xn", [2048, 2], bf16, kind="ExternalInput")
    w1l1_e = nc.dram_tensor("w1l1", [2, HID], f32, kind="ExternalInput")
    wpack_e = nc.dram_tensor("wpack", [HID, WP_COLS], f32, kind="ExternalInput")
    paug_e = nc.dram_tensor("paug", [2048, NJ + 1], f32, kind="ExternalInput")
    headw_e = nc.dram_tensor("headw", [HACT, 4], f32, kind="ExternalInput")
    maskval_e = nc.dram_tensor("maskval", [1, NJ], f32, kind="ExternalInput")
    ident_e = nc.dram_tensor("ident", [HID, HID], f32, kind="ExternalInput")
    out_e = nc.dram_tensor("out", [1, NJ + 1], f32, kind="ExternalOutput")

    with tile.TileContext(nc) as tc:
        with (
            tc.tile_pool(name="sb", bufs=1) as sb,
            tc.tile_pool(name="ps", bufs=1, space="PSUM") as ps,
            tc.tile_pool(name="dr", bufs=1, space="DRAM") as dr,
        ):
            for rep in range(reps):
                _emit_one(nc, sb, ps, dr, adjT_e, xn_e, w1l1_e, wpack_e,
                          paug_e, headw_e, maskval_e, ident_e, out_e, bc2)
    nc.compile()
    return nc


def _emit_one(nc, sb, ps, dr, adjT_e, xn_e, w1l1_e, wpack_e,
              paug_e, headw_e, maskval_e, ident_e, out_e, bc2):
    # ---------------- input DMAs ----------------
    adjT = sb.tile([128, NT * N], bf16, tag="adjT")
    nc.sync.dma_start(
        adjT[0:128, 0:14 * N].rearrange("p (t n) -> p t n", n=N),
        adjT_e[0:1792, :].rearrange("(t p) n -> p t n", p=128))
    nc.sync.dma_start(adjT[0:128, 14 * N:15 * N], adjT_e[1792:1920, :])
    nc.sync.dma_start(adjT[0:80, 15 * N:16 * N], adjT_e[1920:2000, :])

    xn = sb.tile([128, NT * 2], bf16, tag="xn")
    nc.sync.dma_start(xn[:, :].rearrange("p (t c) -> p t c", c=2),
                      xn_e[:, :].rearrange("(t p) c -> p t c", p=128))
    w1l1 = sb.tile([2, HID], f32, tag="w1l1")
    nc.sync.dma_start(w1l1[:, :], w1l1_e[:, :])
    wp = sb.tile([HID, WP_COLS], f32, tag="wp")
    nc.sync.dma_start(wp[:, :], wpack_e[:, :])
    paug = sb.tile([128, NT * (NJ + 1)], f32, tag="paug")
    nc.sync.dma_start(
        paug[:, :].rearrange("p (t j) -> p t j", j=NJ + 1),
        paug_e[:, :].rearrange("(t p) j -> p t j", p=128))
    headw = sb.tile([HACT, 4], f32, tag="headw")
    nc.sync.dma_start(headw[:, :], headw_e[:, :])
    maskval = sb.tile([1, NJ], f32, tag="maskval")
    nc.sync.dma_start(maskval[:, :], maskval_e[:, :])
    ident = sb.tile([HID, HID], f32, tag="ident")
    nc.sync.dma_start(ident[:, :], ident_e[:, :])

    sq = sb.tile([HID, 2048], f32, tag="sq")
    h1m = sb.tile([HID, N], f32, tag="h1m")
    h1 = sb.tile([HID, N], f32, tag="h1")
    h2m = sb.tile([HID, N], f32, tag="h2m")
    h2 = sb.tile([HID, N], f32, tag="h2")
    u2 = sb.tile([128, NT * HID], bf16, tag="u2")
    h2n = sb.tile([128, NT * HID], f32, tag="h2n")

    eps_ap = wp[:, WP_EPS:WP_EPS + 1]

    # ---------------- batchnorm block ----------------
    def bn_block(i, zp, out_sb):
        """out_sb = relu((z - mean)*g*rsqrt(var+eps) + be), stats global."""
        stats2 = sb.tile([HID, 2], f32, tag=f"st2_{i}")
        nc.vector.reduce_sum(stats2[:, 0:1], zp[0:HID, 0:N], axis=AX)
        nc.scalar.activation(sq[:, 0:N], zp[0:HID, 0:N], ACT.Square,
                             accum_out=stats2[:, 1:2])
        cc_in = dr.tile([HID, 2], f32, tag=f"ccin_{i}")
        cc_out = dr.tile([B, HID, 2], f32, tag=f"ccout_{i}")
        nc.sync.dma_start(cc_in[:, :], stats2[:, :])
        nc.gpsimd.collective_compute(
            "AllGather", ALU.bypass,
            replica_groups=[list(range(B))],
            ins=[cc_in.opt()], outs=[cc_out.opt()])
        ag = sb.tile([HID, B, 2], f32, tag=f"ag_{i}")
        nc.sync.dma_start(ag[:, :, :],
                          cc_out[:, :, :].rearrange("r p f -> p r f"))
        w = sb.tile([HID, 8], f32, tag=f"bnv_{i}")
        # (S, Q) = sum over the 8 ranks (strided reduce over rank axis)
        nc.vector.reduce_sum(w[:, 0:2],
                             ag[:, :, :].rearrange("p r f -> p f r"), axis=AX)
        # mean^2 = (S/CNT)^2
        nc.scalar.activation(w[:, 2:3], w[:, 0:1], ACT.Square, scale=1.0 / CNT)
        # var = Q/CNT - mean^2   (biased variance, matches reference)
        nc.vector.tensor_scalar(w[:, 3:4], w[:, 1:2],
                                scalar1=1.0 / CNT, scalar2=w[:, 2:3],
                                op0=ALU.mult, op1=ALU.subtract)
        nc.vector.tensor_scalar_add(w[:, 4:5], w[:, 3:4], eps_ap)
        nc.vector.reciprocal(w[:, 5:6], w[:, 4:5])
        nc.scalar.sqrt(w[:, 6:7], w[:, 5:6])                    # rsqrt(var+eps)
        g_col = wp[:, WP_GBE + 2 * i:WP_GBE + 2 * i + 1]
        be_col = wp[:, WP_GBE + 2 * i + 1:WP_GBE + 2 * i + 2]
        nc.vector.tensor_mul(w[:, 6:7], w[:, 6:7], g_col)       # scale
        # bias = be - (S/CNT)*scale
        nc.vector.tensor_scalar(w[:, 7:8], w[:, 0:1],
                                scalar1=w[:, 6:7], scalar2=1.0 / CNT,
                                op0=ALU.mult, op1=ALU.mult)
        nc.vector.tensor_sub(w[:, 7:8], be_col, w[:, 7:8])
        nc.scalar.activation(out_sb[:, 0:N], zp[0:HID, 0:N], ACT.Relu,
                             bias=w[:, 7:8], scale=w[:, 6:7])

    # ---------------- layer 1: pooled = adj @ x ----------------
    pb = ps.tile([128, 2048], f32, tag="pb")
    for k in range(NT):
        kk = K_LIST[k]
        for (c0, ln) in CHUNKS:
            nc.tensor.matmul(pb[0:2, c0:c0 + ln],
                             lhsT=xn[0:kk, 2 * k:2 * k + 2],
                             rhs=adjT[0:kk, N * k + c0:N * k + c0 + ln],
                             start=(k == 0), stop=(k == NT - 1),
                             skip_group_check=True)
    pooled = sb.tile([2, N], f32, tag="pooled")
    nc.vector.tensor_copy(pooled[:, :], pb[0:2, 0:N])
    # rep1 = pooled^T W1  (channel-major [64, N])
    pa = ps.tile([128, 2048], f32, tag="pa")
    for (c0, ln) in CHUNKS:
        nc.tensor.matmul(pa[0:HID, c0:c0 + ln], lhsT=w1l1[:, :],
                         rhs=pooled[:, c0:c0 + ln], start=True, stop=True)
    bn_block(0, pa, h1m)

    # ---------------- rep2 = h1m @ W2l1 ----------------
    pb = ps.tile([128, 2048], f32, tag="pb")
    for (c0, ln) in CHUNKS:
        nc.tensor.matmul(pb[0:HID, c0:c0 + ln],
                         lhsT=wp[:, WP_W2L1:WP_W2L1 + HID],
                         rhs=h1m[:, c0:c0 + ln], start=True, stop=True)
    bn_block(1, pb, h1)

    # ---------------- u2 = h1 @ W1l2 (node-major, bf16) ----------------
    pa = ps.tile([128, 2048], f32, tag="pa")
    for k in range(NT):
        kk = K_LIST[k]
        nc.tensor.matmul(pa[0:kk, HID * k:HID * (k + 1)],
                         lhsT=h1[0:HID, 128 * k:128 * k + kk],
                         rhs=wp[:, WP_W1L2:WP_W1L2 + HID],
                         start=True, stop=True)
    nc.vector.tensor_copy(u2[:, :], pa[0:128, 0:NT * HID])

    # ---------------- layer 2: z2 = adj @ u2 ----------------
    pb = ps.tile([128, 2048], f32, tag="pb")
    for k in range(NT):
        kk = K_LIST[k]
        for (c0, ln) in CHUNKS:
            nc.tensor.matmul(pb[0:HID, c0:c0 + ln],
                             lhsT=u2[0:kk, HID * k:HID * (k + 1)],
                             rhs=adjT[0:kk, N * k + c0:N * k + c0 + ln],
                             start=(k == 0), stop=(k == NT - 1),
                             skip_group_check=True)
    bn_block(2, pb, h2m)

    # ---------------- rep2l2 = h2m @ W2l2 ----------------
    pa = ps.tile([128, 2048], f32, tag="pa")
    for (c0, ln) in CHUNKS:
        nc.tensor.matmul(pa[0:HID, c0:c0 + ln],
                         lhsT=wp[:, WP_W2L2:WP_W2L2 + HID],
                         rhs=h2m[:, c0:c0 + ln], start=True, stop=True)
    bn_block(3, pa, h2)

    # ---------------- transpose h2 -> node-major ----------------
    pb = ps.tile([128, 2048], f32, tag="pb")
    for k in range(NT):
        kk = K_LIST[k]
        nc.tensor.transpose(pb[0:kk, HID * k:HID * (k + 1)],
                            h2[0:HID, 128 * k:128 * k + kk],
                            ident[:, :])
    nc.vector.tensor_copy(h2n[:, :], pb[0:128, 0:NT * HID])

    # ---------------- candidates + pooling: C = h2n^T @ paug ----------------
    pa = ps.tile([128, 2048], f32, tag="pa")
    for k in range(NT):
        kk = K_LIST[k]
        nc.tensor.matmul(pa[0:HID, 0:NJ + 1],
                         lhsT=h2n[0:kk, HID * k:HID * (k + 1)],
                         rhs=paug[0:kk, (NJ + 1) * k:(NJ + 1) * (k + 1)],
                         start=(k == 0), stop=(k == NT - 1))
    C = sb.tile([HID, NJ + 1], f32, tag="C")
    nc.vector.tensor_copy(C[:, :], pa[0:HID, 0:NJ + 1])

    # ---------------- actor / critic heads ----------------
    pb2 = ps.tile([128, 2048], f32, tag="pb")
    nc.tensor.matmul(pb2[0:HACT, 0:NJ + 1],
                     lhsT=wp[:, WP_ACT:WP_ACT + HACT],
                     rhs=C[:, :], start=True, stop=True)
    nc.tensor.matmul(pb2[0:HACT, 512:513],
                     lhsT=wp[:, WP_ACT + HACT:WP_ACT + 2 * HACT],
                     rhs=C[:, NJ:NJ + 1], start=True, stop=True)
    nc.tensor.matmul(pb2[0:HACT, 513:514],
                     lhsT=wp[:, WP_ACT + 2 * HACT:WP_ACT + 3 * HACT],
                     rhs=C[:, NJ:NJ + 1], start=True, stop=True)
    hw = sb.tile([HACT, 2], f32, tag="hw")
    nc.vector.tensor_add(hw[:, 0:1], pb2[0:HACT, 512:513], headw[:, 2:3])
    T = sb.tile([HACT, NJ + 1], f32, tag="T")
    nc.scalar.activation(T[:, 0:NJ], pb2[0:HACT, 0:NJ], ACT.Tanh,
                         bias=hw[:, 0:1])
    nc.scalar.activation(T[:, NJ:NJ + 1], pb2[0:HACT, 513:514], ACT.Tanh,
                         bias=headw[:, 3:4])
    nc.tensor.matmul(pb2[0:1, 1024:1024 + NJ], lhsT=headw[:, 0:1],
                     rhs=T[:, 0:NJ], start=True, stop=True)
    nc.tensor.matmul(pb2[0:1, 1536:1537], lhsT=headw[:, 1:2],
                     rhs=T[:, NJ:NJ + 1], start=True, stop=True)
    pit = sb.tile([1, NJ + 1], f32, tag="pit")
    sm = sb.tile([1, NJ], f32, tag="sm")
    nc.vector.tensor_add(sm[:, :], pb2[0:1, 1024:1024 + NJ], maskval[:, :])
    red = sb.tile([1, 4], f32, tag="red")
    nc.vector.reduce_max(red[:, 0:1], sm[:, :], axis=AX)
    nc.vector.tensor_scalar_mul(red[:, 1:2], red[:, 0:1], -1.0)
    e = sb.tile([1, NJ], f32, tag="e")
    nc.scalar.activation(e[:, :], sm[:, :], ACT.Exp, bias=red[:, 1:2],
                         accum_out=red[:, 2:3])
    nc.vector.reciprocal(red[:, 3:4], red[:, 2:3])
    nc.scalar.activation(pit[:, 0:NJ], e[:, :], ACT.Copy, scale=red[:, 3:4])
    nc.scalar.activation(pit[:, NJ:NJ + 1], pb2[0:1, 1536:1537],
                         ACT.Copy, bias=float(bc2))
    nc.sync.dma_start(out_e[:, :], pit[:, :])


# ---------------- host side ----------------

def _prep_in_maps(x, graph_pool, adj, candidate, mask, params):
    x = np.asarray(x, dtype=np.float32)
    graph_pool = np.asarray(graph_pool, dtype=np.float32)
    adj = np.asarray(adj, dtype=np.float32)
    candidate = np.asarray(candidate).astype(np.int64)
    mask = np.asarray(mask)
    g0, g1 = params['gin'][0], params['gin'][1]
    a, c = params['actor'], params['critic']
    f = lambda t: np.ascontiguousarray(np.asarray(t, dtype=np.float32))

    w1l1 = f(g0['W1'])
    gbe = np.stack([f(g0['g1']), f(g0['be1']), f(g0['g']), f(g0['be']),
                    f(g1['g1']), f(g1['be1']), f(g1['g']), f(g1['be'])],
                   axis=1)  # [64, 8]
    wa1 = f(a['W1'])                       # [128, 32]
    wpack = np.concatenate(
        [f(g0['W2']), f(g1['W1']), f(g1['W2']), gbe,
         np.full((HID, 1), BN_EPS, np.float32),
         wa1[:HID], wa1[HID:], f(c['W1'])], axis=1)
    assert wpack.shape == (HID, WP_COLS)
    headw = np.stack([f(a['W2'])[:, 0], f(c['W2'])[:, 0],
                      np.broadcast_to(f(a['b1']), (HACT,)),
                      np.broadcast_to(f(c['b1']), (HACT,))], axis=1)
    ba2 = float(np.asarray(a['b2']).reshape(-1)[0])
    bc2 = float(np.asarray(c['b2']).reshape(-1)[0])
    ident = np.eye(HID, dtype=np.float32)

    in_maps = []
    for b in range(B):
        sl = slice(b * N, (b + 1) * N)
        adjT = np.ascontiguousarray(adj[sl, sl].T).astype(ml_dtypes.bfloat16)
        xn = np.zeros((2048, 2), dtype=ml_dtypes.bfloat16)
        xn[0:N] = x[sl].astype(ml_dtypes.bfloat16)
        paug = np.zeros((2048, NJ + 1), dtype=np.float32)
        paug[candidate[b], np.arange(NJ)] = 1.0
        paug[0:N, NJ] = graph_pool[b, sl]
        maskval = (np.where(mask[b], NEG_BIG, 0.0) + ba2) \
            .astype(np.float32).reshape(1, NJ)
        in_maps.append({
            "adjT": adjT, "xn": xn, "w1l1": w1l1, "wpack": wpack,
            "paug": paug, "headw": headw, "maskval": maskval, "ident": ident,
        })
    return in_maps, bc2


def run(inputs: dict, reps: int = 1, nc=None):
    """Run on hardware; returns ((pi, v), nc) so callers can reuse the graph."""
    in_maps, bc2 = _prep_in_maps(
        inputs['x'], inputs['graph_pool'], inputs['adj'],
        inputs['candidate'], inputs['mask'], inputs['params'])
    if nc is None:
        nc = build_graph(bc2, reps=reps)
    res = run_bass_kernel_spmd(nc, in_maps, core_ids=list(range(B)))
    outs = [res.results[i]["out"] for i in range(B)]
    pi = np.stack([o[0, :NJ] for o in outs]).astype(np.float32)[:, :, None]
    v = np.stack([o[0, NJ:NJ + 1] for o in outs]).astype(np.float32)
    return (pi, v), nc


def kernel(x, graph_pool, padded_nei, adj, candidate, mask, params):
    (pi, v), _ = run({'x': x, 'graph_pool': graph_pool, 'adj': adj,
                      'candidate': candidate, 'mask': mask, 'params': params})
    return pi, v


# revision 7
# speedup vs baseline: 1.6547x; 1.1347x over previous
"""Trainium2 Bass kernel for nn_ActorCritic (GIN actor-critic, 8 disjoint graphs).

Sharding: graph b -> NeuronCore b (data parallel over the batch of disjoint
graphs). Each core holds its diagonal adjacency block (transposed, bf16),
its node features, and replicated MLP weights. The only cross-core traffic
is the 4 BatchNorm statistics exchanges, done as tiny AllReduces.

Host side only reshapes / transposes / one-hot-encodes inputs into the exact
SBUF layouts (no model math on CPU), so every input DMA is contiguous.
"""

import numpy as np
import ml_dtypes

import concourse.bass as bass
import concourse.bacc as bacc
import concourse.mybir as mybir
import concourse.tile as tile
from concourse.bass_utils import run_bass_kernel_spmd

# ---- problem constants ----
B = 8            # graphs == cores
N = 2000         # nodes per graph
NJ = 100         # candidates per graph
HID = 64
HACT = 32
BN_EPS = 1e-5
CNT = float(B * N)          # batchnorm count (16000)
NEG_BIG = -1.0e30

NT = 16                      # node k-tiles of 128 (last has 80 rows)
K_LIST = [128] * 15 + [80]
CHUNKS = [(0, 512), (512, 512), (1024, 512), (1536, 464)]  # node columns

f32 = mybir.dt.float32
bf16 = mybir.dt.bfloat16

AX = mybir.AxisListType.X
ALU = mybir.AluOpType
ACT = mybir.ActivationFunctionType

# wpack column layout: [w2l1 64][w1l2 64][w2l2 64][gbe 8][eps 1][actw 96]
WP_W2L1 = 0
WP_W1L2 = 64
WP_W2L2 = 128
WP_GBE = 192
WP_EPS = 200
WP_ACT = 201
WP_COLS = 297


def build_graph(bc2: float, reps: int = 1):
    nc = bacc.Bacc("TRN2", target_bir_lowering=False, debug=False,
                   num_devices=B)

    # all pre-shuffled on host to the exact SBUF layout (contiguous DMAs)
    adjT_e = nc.dram_tensor("adjT", [128, NT * N], bf16, kind="ExternalInput")
    xn_e = nc.dram_tensor("xn", [128, NT * 2], bf16, kind="ExternalInput")
    w1l1_e = nc.dram_tensor("w1l1", [2, HID], f32, kind="ExternalInput")
    wpack_e = nc.dram_tensor("wpack", [HID, WP_COLS], f32, kind="ExternalInput")
    paug_e = nc.dram_tensor("paug", [128, NT * (NJ + 1)], f32,
                            kind="ExternalInput")
    headw_e = nc.dram_tensor("headw", [HACT, 4], f32, kind="ExternalInput")
    maskval_e = nc.dram_tensor("maskval", [1, NJ], f32, kind="ExternalInput")
    ident_e = nc.dram_tensor("ident", [HID, HID], f32, kind="ExternalInput")
    out_e = nc.dram_tensor("out", [1, NJ + 1], f32, kind="ExternalOutput")

    with tile.TileContext(nc) as tc:
        with (
            tc.tile_pool(name="sb", bufs=1) as sb,
            tc.tile_pool(name="ps", bufs=1, space="PSUM") as ps,
            tc.tile_pool(name="dr", bufs=1, space="DRAM") as dr,
        ):
            for rep in range(reps):
                _emit_one(nc, sb, ps, dr, adjT_e, xn_e, w1l1_e, wpack_e,
                          paug_e, headw_e, maskval_e, ident_e, out_e, bc2)
    nc.compile()
    return nc


def _emit_one(nc, sb, ps, dr, adjT_e, xn_e, w1l1_e, wpack_e,
              paug_e, headw_e, maskval_e, ident_e, out_e, bc2):
    # ---------------- input DMAs (all contiguous) ----------------
    adjT = sb.tile([128, NT * N], bf16, tag="adjT")
    nc.sync.dma_start(adjT[:, :], adjT_e[:, :])
    xn = sb.tile([128, NT * 2], bf16, tag="xn")
    nc.sync.dma_start(xn[:, :], xn_e[:, :])
    w1l1 = sb.tile([2, HID], f32, tag="w1l1")
    nc.sync.dma_start(w1l1[:, :], w1l1_e[:, :])
    wp = sb.tile([HID, WP_COLS], f32, tag="wp")
    nc.sync.dma_start(wp[:, :], wpack_e[:, :])
    paug = sb.tile([128, NT * (NJ + 1)], f32, tag="paug")
    nc.sync.dma_start(paug[:, :], paug_e[:, :])
    headw = sb.tile([HACT, 4], f32, tag="headw")
    nc.sync.dma_start(headw[:, :], headw_e[:, :])
    maskval = sb.tile([1, NJ], f32, tag="maskval")
    nc.sync.dma_start(maskval[:, :], maskval_e[:, :])
    ident = sb.tile([HID, HID], f32, tag="ident")
    nc.sync.dma_start(ident[:, :], ident_e[:, :])

    sq = sb.tile([HID, 2048], f32, tag="sq")
    h1m = sb.tile([HID, N], f32, tag="h1m")
    h1 = sb.tile([HID, N], f32, tag="h1")
    h2m = sb.tile([HID, N], f32, tag="h2m")
    h2 = sb.tile([HID, N], f32, tag="h2")
    u2 = sb.tile([128, NT * HID], bf16, tag="u2")
    h2n = sb.tile([128, NT * HID], f32, tag="h2n")

    eps_ap = wp[:, WP_EPS:WP_EPS + 1]

    # ---------------- batchnorm block ----------------
    def bn_block(i, zp, out_sb):
        """out_sb = relu((z - mean)*g*rsqrt(var+eps) + be), stats global.

        Stats on DVE, then one AllReduce of [64, 2] (sum, sumsq), then the
        scale/bias chain entirely on the scalar engine (no cross-engine sync).
        """
        stats2 = sb.tile([HID, 2], f32, tag=f"st2_{i}")
        nc.vector.reduce_sum(stats2[:, 0:1], zp[0:HID, 0:N], axis=AX)
        nc.scalar.activation(sq[:, 0:N], zp[0:HID, 0:N], ACT.Square,
                             accum_out=stats2[:, 1:2])
        cc_in = dr.tile([HID, 2], f32, tag=f"ccin_{i}")
        cc_out = dr.tile([HID, 2], f32, tag=f"ccout_{i}")
        nc.sync.dma_start(cc_in[:, :], stats2[:, :])
        nc.gpsimd.collective_compute(
            "AllReduce", ALU.add,
            replica_groups=[list(range(B))],
            ins=[cc_in.opt()], outs=[cc_out.opt()])
        g_sb = sb.tile([HID, 2], f32, tag=f"g_{i}")
        nc.sync.dma_start(g_sb[:, :], cc_out[:, :])
        w = sb.tile([HID, 8], f32, tag=f"bnv_{i}")
        S, Q = g_sb[:, 0:1], g_sb[:, 1:2]
        # chain on scalar engine: out = func(in*scale + bias)
        nc.scalar.activation(w[:, 0:1], S, ACT.Square, scale=1.0 / CNT)  # m^2
        nc.scalar.activation(w[:, 1:2], Q, ACT.Identity, scale=1.0 / CNT,
                             bias=eps_ap)                      # Q/CNT + eps
        nc.scalar.activation(w[:, 2:3], w[:, 0:1], ACT.Identity, scale=-1.0,
                             bias=w[:, 1:2])                   # var + eps
        nc.scalar.activation(w[:, 3:4], w[:, 2:3],
                             ACT.Abs_reciprocal_sqrt)          # rsqrt(var+eps)
        g_col = wp[:, WP_GBE + 2 * i:WP_GBE + 2 * i + 1]
        be_col = wp[:, WP_GBE + 2 * i + 1:WP_GBE + 2 * i + 2]
        nc.scalar.activation(w[:, 4:5], w[:, 3:4], ACT.Identity,
                             scale=g_col)                      # scale = g*rsq
        nc.scalar.activation(w[:, 5:6], S, ACT.Identity,
                             scale=w[:, 4:5])                  # S*scale
        nc.scalar.activation(w[:, 6:7], w[:, 5:6], ACT.Identity,
                             scale=-1.0 / CNT, bias=be_col)    # bias
        nc.scalar.activation(out_sb[:, 0:N], zp[0:HID, 0:N], ACT.Relu,
                             bias=w[:, 6:7], scale=w[:, 4:5])

    # ---------------- layer 1: pooled = adj @ x ----------------
    pb = ps.tile([128, 2048], f32, tag="pb")
    for k in range(NT):
        kk = K_LIST[k]
        for (c0, ln) in CHUNKS:
            nc.tensor.matmul(pb[0:2, c0:c0 + ln],
                             lhsT=xn[0:kk, 2 * k:2 * k + 2],
                             rhs=adjT[0:kk, N * k + c0:N * k + c0 + ln],
                             start=(k == 0), stop=(k == NT - 1),
                             skip_group_check=True)
    pooled = sb.tile([2, N], f32, tag="pooled")
    nc.vector.tensor_copy(pooled[:, :], pb[0:2, 0:N])
    # rep1 = pooled^T W1  (channel-major [64, N])
    pa = ps.tile([128, 2048], f32, tag="pa")
    for (c0, ln) in CHUNKS:
        nc.tensor.matmul(pa[0:HID, c0:c0 + ln], lhsT=w1l1[:, :],
                         rhs=pooled[:, c0:c0 + ln], start=True, stop=True)
    bn_block(0, pa, h1m)

    # ---------------- rep2 = h1m @ W2l1 ----------------
    pb = ps.tile([128, 2048], f32, tag="pb")
    for (c0, ln) in CHUNKS:
        nc.tensor.matmul(pb[0:HID, c0:c0 + ln],
                         lhsT=wp[:, WP_W2L1:WP_W2L1 + HID],
                         rhs=h1m[:, c0:c0 + ln], start=True, stop=True)
    bn_block(1, pb, h1)

    # ---------------- u2 = h1 @ W1l2 (node-major, bf16) ----------------
    pa = ps.tile([128, 2048], f32, tag="pa")
    for k in range(NT):
        kk = K_LIST[k]
        nc.tensor.matmul(pa[0:kk, HID * k:HID * (k + 1)],
                         lhsT=h1[0:HID, 128 * k:128 * k + kk],
                         rhs=wp[:, WP_W1L2:WP_W1L2 + HID],
                         start=True, stop=True)
    nc.vector.tensor_copy(u2[:, :], pa[0:128, 0:NT * HID])

    # ---------------- layer 2: z2 = adj @ u2 ----------------
    pb = ps.tile([128, 2048], f32, tag="pb")
    for k in range(NT):
        kk = K_LIST[k]
        for (c0, ln) in CHUNKS:
            nc.tensor.matmul(pb[0:HID, c0:c0 + ln],
                             lhsT=u2[0:kk, HID * k:HID * (k + 1)],
                             rhs=adjT[0:kk, N * k + c0:N * k + c0 + ln],
                             start=(k == 0), stop=(k == NT - 1),
                             skip_group_check=True)
    bn_block(2, pb, h2m)

    # ---------------- rep2l2 = h2m @ W2l2 ----------------
    pa = ps.tile([128, 2048], f32, tag="pa")
    for (c0, ln) in CHUNKS:
        nc.tensor.matmul(pa[0:HID, c0:c0 + ln],
                         lhsT=wp[:, WP_W2L2:WP_W2L2 + HID],
                         rhs=h2m[:, c0:c0 + ln], start=True, stop=True)
    bn_block(3, pa, h2)

    # ---------------- transpose h2 -> node-major ----------------
    pb = ps.tile([128, 2048], f32, tag="pb")
    for k in range(NT):
        kk = K_LIST[k]
        nc.tensor.transpose(pb[0:kk, HID * k:HID * (k + 1)],
                            h2[0:HID, 128 * k:128 * k + kk],
                            ident[:, :])
    nc.vector.tensor_copy(h2n[:, :], pb[0:128, 0:NT * HID])

    # ---------------- candidates + pooling: C = h2n^T @ paug ----------------
    pa = ps.tile([128, 2048], f32, tag="pa")
    for k in range(NT):
        kk = K_LIST[k]
        nc.tensor.matmul(pa[0:HID, 0:NJ + 1],
                         lhsT=h2n[0:kk, HID * k:HID * (k + 1)],
                         rhs=paug[0:kk, (NJ + 1) * k:(NJ + 1) * (k + 1)],
                         start=(k == 0), stop=(k == NT - 1))
    C = sb.tile([HID, NJ + 1], f32, tag="C")
    nc.vector.tensor_copy(C[:, :], pa[0:HID, 0:NJ + 1])

    # ---------------- actor / critic heads ----------------
    pb2 = ps.tile([128, 2048], f32, tag="pb")
    nc.tensor.matmul(pb2[0:HACT, 0:NJ + 1],
                     lhsT=wp[:, WP_ACT:WP_ACT + HACT],
                     rhs=C[:, :], start=True, stop=True)
    nc.tensor.matmul(pb2[0:HACT, 512:513],
                     lhsT=wp[:, WP_ACT + HACT:WP_ACT + 2 * HACT],
                     rhs=C[:, NJ:NJ + 1], start=True, stop=True)
    nc.tensor.matmul(pb2[0:HACT, 513:514],
                     lhsT=wp[:, WP_ACT + 2 * HACT:WP_ACT + 3 * HACT],
                     rhs=C[:, NJ:NJ + 1], start=True, stop=True)
    hw = sb.tile([HACT, 2], f32, tag="hw")
    nc.vector.tensor_add(hw[:, 0:1], pb2[0:HACT, 512:513], headw[:, 2:3])
    T = sb.tile([HACT, NJ + 1], f32, tag="T")
    nc.scalar.activation(T[:, 0:NJ], pb2[0:HACT, 0:NJ], ACT.Tanh,
                         bias=hw[:, 0:1])
    nc.scalar.activation(T[:, NJ:NJ + 1], pb2[0:HACT, 513:514], ACT.Tanh,
                         bias=headw[:, 3:4])
    nc.tensor.matmul(pb2[0:1, 1024:1024 + NJ], lhsT=headw[:, 0:1],
                     rhs=T[:, 0:NJ], start=True, stop=True)
    nc.tensor.matmul(pb2[0:1, 1536:1537], lhsT=headw[:, 1:2],
                     rhs=T[:, NJ:NJ + 1], start=True, stop=True)
    pit = sb.tile([1, NJ + 1], f32, tag="pit")
    sm = sb.tile([1, NJ], f32, tag="sm")
    # scores are tanh-bounded (|s| < ~6), so exp() without max-subtraction
    # is safe; masked entries carry -1e30 -> exp underflows to exactly 0.
    nc.vector.tensor_add(sm[:, :], pb2[0:1, 1024:1024 + NJ], maskval[:, :])
    red = sb.tile([1, 2], f32, tag="red")
    e = sb.tile([1, NJ], f32, tag="e")
    nc.scalar.activation(e[:, :], sm[:, :], ACT.Exp, accum_out=red[:, 0:1])
    nc.vector.reciprocal(red[:, 1:2], red[:, 0:1])
    nc.vector.tensor_scalar_mul(pit[:, 0:NJ], e[:, :], red[:, 1:2])
    nc.scalar.activation(pit[:, NJ:NJ + 1], pb2[0:1, 1536:1537],
                         ACT.Copy, bias=float(bc2))
    nc.sync.dma_start(out_e[:, :], pit[:, :])


# ---------------- host side ----------------

def _shuffle_tiles(arr2d, cols):
    """[2000+, cols] node-major -> [128, NT*cols] SBUF tile layout."""
    out = np.zeros((128, NT * cols), dtype=arr2d.dtype)
    for t in range(NT):
        rows = arr2d[128 * t:128 * (t + 1)]
        out[0:rows.shape[0], cols * t:cols * t + cols] = rows
    return out


def _prep_in_maps(x, graph_pool, adj, candidate, mask, params):
    x = np.asarray(x, dtype=np.float32)
    graph_pool = np.asarray(graph_pool, dtype=np.float32)
    adj = np.asarray(adj, dtype=np.float32)
    candidate = np.asarray(candidate).astype(np.int64)
    mask = np.asarray(mask)
    g0, g1 = params['gin'][0], params['gin'][1]
    a, c = params['actor'], params['critic']
    f = lambda t: np.ascontiguousarray(np.asarray(t, dtype=np.float32))

    w1l1 = f(g0['W1'])
    gbe = np.stack([f(g0['g1']), f(g0['be1']), f(g0['g']), f(g0['be']),
                    f(g1['g1']), f(g1['be1']), f(g1['g']), f(g1['be'])],
                   axis=1)  # [64, 8]
    wa1 = f(a['W1'])                       # [128, 32]
    wpack = np.concatenate(
        [f(g0['W2']), f(g1['W1']), f(g1['W2']), gbe,
         np.full((HID, 1), BN_EPS, np.float32),
         wa1[:HID], wa1[HID:], f(c['W1'])], axis=1)
    assert wpack.shape == (HID, WP_COLS)
    headw = np.stack([f(a['W2'])[:, 0], f(c['W2'])[:, 0],
                      np.broadcast_to(f(a['b1']), (HACT,)),
                      np.broadcast_to(f(c['b1']), (HACT,))], axis=1)
    ba2 = float(np.asarray(a['b2']).reshape(-1)[0])
    bc2 = float(np.asarray(c['b2']).reshape(-1)[0])
    ident = np.eye(HID, dtype=np.float32)

    in_maps = []
    for b in range(B):
        sl = slice(b * N, (b + 1) * N)
        adjT = np.ascontiguousarray(adj[sl, sl].T)
        adjT = _shuffle_tiles(adjT, N).astype(ml_dtypes.bfloat16)
        xn = _shuffle_tiles(x[sl], 2).astype(ml_dtypes.bfloat16)
        paug = np.zeros((N, NJ + 1), dtype=np.float32)
        paug[candidate[b], np.arange(NJ)] = 1.0
        paug[:, NJ] = graph_pool[b, sl]
        paug = _shuffle_tiles(paug, NJ + 1)
        maskval = (np.where(mask[b], NEG_BIG, 0.0) + ba2) \
            .astype(np.float32).reshape(1, NJ)
        in_maps.append({
            "adjT": adjT, "xn": xn, "w1l1": w1l1, "wpack": wpack,
            "paug": paug, "headw": headw, "maskval": maskval, "ident": ident,
        })
    return in_maps, bc2


def run(inputs: dict, reps: int = 1, nc=None):
    """Run on hardware; returns ((pi, v), nc) so callers can reuse the graph."""
    in_maps, bc2 = _prep_in_maps(
        inputs['x'], inputs['graph_pool'], inputs['adj'],
        inputs['candidate'], inputs['mask'], inputs['params'])
    if nc is None:
        nc = build_graph(bc2, reps=reps)
    res = run_bass_kernel_spmd(nc, in_maps, core_ids=list(range(B)))
    outs = [res.results[i]["out"] for i in range(B)]
    pi = np.stack([o[0, :NJ] for o in outs]).astype(np.float32)[:, :, None]
    v = np.stack([o[0, NJ:NJ + 1] for o in outs]).astype(np.float32)
    return (pi, v), nc


def kernel(x, graph_pool, padded_nei, adj, candidate, mask, params):
    (pi, v), _ = run({'x': x, 'graph_pool': graph_pool, 'adj': adj,
                      'candidate': candidate, 'mask': mask, 'params': params})
    return pi, v
